# revision 25
# baseline (speedup 1.0000x reference)
"""Anisotropic upsampling kernel for Trainium2 (8 NeuronCores, batch-sharded).

Computes, for inputs x0 (8,64,64,256), x1 (8,64,128,128), x2 (8,64,256,64):
    out0 = (up_h(x0) + up_w(x1)) / 2   -> (8,64,128,256)
    out1 = (up_h(x1) + up_w(x2)) / 2   -> (8,64,256,128)
where up() is the stride-2, length-5 normalized zero-insert upsampler:
    up(x)[2m]   = (x[m-1]+x[m]+x[m+1])/3   (edges: mean of the 2 valid taps)
    up(x)[2m+1] = (x[m]+x[m+1])/2          (edge m=W-1: x[W-1])

Layout: partitions p = 2*c + ha where ha selects the top/bottom half of the
h range.  Every DRAM transfer is a single 128-partition DMA whose OUTER walk
dim is c (64 entries): the SDMA spray gives each of the 16 engines a
contiguous block of 4 channels = 8 partitions = one SBUF port group, so DMA
runs at the HBM line rate with one contiguous descriptor per partition.

The work is organized as 4 fused chunk pairs (out0 rows 16i..16i+16 and
out1 rows 32i..32i+32 per half): x1's rows serve BOTH as out0's w-branch
input and out1's h-branch input from a single load.  Both up_h and up_w are
free-axis stencils computed in bf16 so VectorE's 2x packing applies
(h-shifts are row-pitch multiples; the w-branch +1 shift is a second
f32-source cast, which is always 4B-aligned).  VectorE does the tap sums
and fused scale-accumulate merges into the fp32 output; ScalarE does the
casts and scaled row-parity copies.  Global h-edge rows use per-partition
scale vectors (ha parity) plus 1-row halo DMAs.
"""

import numpy as np

_NC_CACHE = {}


def _build():
    import concourse.bass as bass
    import concourse.mybir as mybir
    from concourse import bacc
    from concourse.tile import TileContext

    f32 = mybir.dt.float32
    bf16 = mybir.dt.bfloat16
    MUL = mybir.AluOpType.mult
    ADD = mybir.AluOpType.add

    nc = bacc.Bacc("TRN2", target_bir_lowering=False, debug=False, num_devices=8)

    x0_d = nc.dram_tensor("x0", [64, 64, 256], f32, kind="ExternalInput")
    x1_d = nc.dram_tensor("x1", [64, 128, 128], f32, kind="ExternalInput")
    x2_d = nc.dram_tensor("x2", [64, 256, 64], f32, kind="ExternalInput")
    out0 = nc.dram_tensor("out0", [64, 128, 256], f32, kind="ExternalOutput")
    out1 = nc.dram_tensor("out1", [64, 256, 128], f32, kind="ExternalOutput")

    # per-partition (ha-parity) scale vectors for the h-edge fixups
    ha = (np.arange(128) % 2).astype(np.float32)  # p = 2c + ha
    consts = np.stack(
        [0.25 * (1 - ha), (1.0 / 6.0) * ha, (1.0 / 6.0) * (1 - ha), 0.25 * ha],
        axis=1,
    )  # (128, 4)

    def walk(d, shape, lo, n):
        """c-outer (ha c | rows lo..lo+n, w) 4D walk of DRAM d (64, HH, W).
        lo is a within-half row index; partition order p = 2c + ha."""
        C, HH, W = shape
        return bass.AP(
            d, lo * W,
            [[HH * W, C], [(HH // 2) * W, 2], [W, n], [1, W]],
        )

    def halo_row(d, shape, row0, row1):
        """1-row 128-partition walk: ha=0 partitions read global row0,
        ha=1 read global row1."""
        C, HH, W = shape
        return bass.AP(
            d, row0 * W,
            [[HH * W, C], [(row1 - row0) * W, 2], [1, W]],
        )

    with TileContext(nc) as tc:
        with (
            tc.tile_pool(name="cpool", bufs=1) as cpool,
            tc.tile_pool(name="inpool", bufs=3) as inpool,
            tc.tile_pool(name="bfpool", bufs=2) as bfpool,
            tc.tile_pool(name="stpool", bufs=2) as stpool,
            tc.tile_pool(name="opool", bufs=2) as opool,
        ):
            cv_d = nc.inline_tensor(consts, "edge_scales")
            cv = cpool.tile([128, 4], f32, tag="cv")
            nc.sync.dma_start(out=cv, in_=cv_d[:, :])

            def load_xv(d, shape, H, nh, m0, first, last, tag, nh_max):
                """Halo'd h-branch source load: nh+2 rows per half starting
                at within-half source row m0-1, with the global edge tricks
                on the first/last chunks."""
                XV = inpool.tile([128, nh_max + 2, shape[2]], f32, tag=tag)
                if first:
                    nc.sync.dma_start(
                        out=XV[:, 1:nh + 2, :], in_=walk(d, shape, 0, nh + 1))
                    nc.sync.dma_start(
                        out=XV[:, 0, :],
                        in_=halo_row(d, shape, 0, H // 2 - 1))
                elif last:
                    nc.sync.dma_start(
                        out=XV[:, 0:nh + 1, :],
                        in_=walk(d, shape, m0 - 1, nh + 1))
                    nc.sync.dma_start(
                        out=XV[:, nh + 1, :],
                        in_=halo_row(d, shape, m0 + nh, H - 1))
                else:
                    nc.sync.dma_start(
                        out=XV[:, 0:nh + 2, :],
                        in_=walk(d, shape, m0 - 1, nh + 2))
                return XV

            def load_pair(r0, R0, first, last):
                XV0 = load_xv(x0_d, (64, 64, 256), 64, R0 // 2, r0 // 2,
                              first, last, "xv0", 8)
                XV1 = load_xv(x1_d, (64, 128, 128), 128, R0, r0,
                              first, last, "xv1", 16)
                XH2 = inpool.tile([128, 32, 64], f32, tag="xh2")
                nc.sync.dma_start(
                    out=XH2[:, 0:2 * R0, :],
                    in_=walk(x2_d, (64, 256, 64), 2 * r0, 2 * R0))
                return XV0, XV1, XH2

            def compute_half(out_d, out_shape, XVb, XHf32, XHb, H, W, R,
                             r0, first, last, otag):
                """One output chunk given bf16 h-source XVb (nh+2 rows) and
                w-source XHb (R rows, bf16) + its f32 original XHf32."""
                nh = R // 2

                # shifted cast from the (4B-aligned) f32 source
                XHs = bfpool.tile([128, R, W - 1], bf16, tag="xhs")
                nc.scalar.copy(XHs, XHf32[:, :, 1:W])

                # ---- stencil sums (VectorE bf16 2x) ----
                SH = stpool.tile([128, nh + 1, 2 * W], bf16, tag="sh")
                nc.vector.tensor_add(
                    SH, XVb[:, 0:nh + 1, :], XVb[:, 1:nh + 2, :])
                TH = stpool.tile([128, nh, 2 * W], bf16, tag="th")
                nc.vector.tensor_add(
                    TH, SH[:, 0:nh, :], XVb[:, 2:nh + 2, :])
                SWE = stpool.tile([128, R, W], bf16, tag="swe")
                nc.vector.tensor_add(
                    SWE[:, :, 0:W - 1], XHb[:, :, 0:W - 1], XHs)
                nc.scalar.mul(
                    SWE[:, :, W - 1:W], XHb[:, :, W - 1:W], 2.0)
                TW = stpool.tile([128, R, W - 2], bf16, tag="tw")
                nc.vector.tensor_add(
                    TW, SWE[:, :, 0:W - 2], XHb[:, :, 2:W])

                # ---- h-branch scaled row-parity writes (ScalarE) ----
                O = opool.tile([128, R, 2 * W], f32, tag="o")
                nc.scalar.mul(O[:, 1:R:2, :], SH[:, 1:nh + 1, :], 0.25)
                nc.scalar.mul(O[:, 0:R:2, :], TH, 1.0 / 6.0)

                # ---- global h-edge row fixups (per-partition scales) ----
                if first:
                    nc.scalar.mul(O[:, 0, :], SH[:, 1, :], cv[:, 0:1])
                    nc.vector.scalar_tensor_tensor(
                        O[:, 0, :], TH[:, 0, :], cv[:, 1:2], O[:, 0, :],
                        op0=MUL, op1=ADD)
                if last:
                    nc.scalar.mul(
                        O[:, R - 2, :], TH[:, nh - 1, :], cv[:, 2:3])
                    nc.vector.scalar_tensor_tensor(
                        O[:, R - 2, :], SH[:, nh - 1, :], cv[:, 3:4],
                        O[:, R - 2, :], op0=MUL, op1=ADD)
                    # row R-1 needs no fixup (doubled-tap halo trick)

                # ---- w-plane fused scale-accumulate merges (VectorE) ----
                nc.vector.scalar_tensor_tensor(
                    O[:, :, 1:2 * W:2], SWE, 0.25, O[:, :, 1:2 * W:2],
                    op0=MUL, op1=ADD)
                nc.vector.scalar_tensor_tensor(
                    O[:, :, 2:2 * W - 2:2], TW, 1.0 / 6.0,
                    O[:, :, 2:2 * W - 2:2], op0=MUL, op1=ADD)
                nc.vector.scalar_tensor_tensor(
                    O[:, :, 0:2 * W - 1:2 * W - 2],
                    SWE[:, :, 0:W - 1:W - 2], 0.25,
                    O[:, :, 0:2 * W - 1:2 * W - 2], op0=MUL, op1=ADD)

                nc.sync.dma_start(
                    out=walk(out_d, out_shape, r0, R), in_=O)

            def compute_pair(r0, R0, first, last, XV0, XV1, XH2):
                nv0, nv1 = R0 // 2 + 2, R0 + 2
                # casts (ScalarE)
                XV0b = bfpool.tile([128, 10, 256], bf16, tag="xv0b")
                nc.scalar.copy(XV0b[:, 0:nv0, :], XV0[:, 0:nv0, :])
                XV1b = bfpool.tile([128, 18, 128], bf16, tag="xv1b")
                nc.scalar.copy(XV1b[:, 0:nv1, :], XV1[:, 0:nv1, :])
                XH2b = bfpool.tile([128, 32, 64], bf16, tag="xh2b")
                nc.scalar.copy(XH2b[:, 0:2 * R0, :], XH2[:, 0:2 * R0, :])

                # out0: h-branch from x0, w-branch from x1 rows r0..r0+R0
                # (= XV1 local rows 1..R0+1, valid for every chunk)
                compute_half(out0, (64, 128, 256), XV0b,
                             XV1[:, 1:R0 + 1, :], XV1b[:, 1:R0 + 1, :],
                             H=64, W=128, R=R0, r0=r0,
                             first=first, last=last, otag="a")
                # out1: h-branch from x1, w-branch from x2
                compute_half(out1, (64, 256, 128), XV1b,
                             XH2[:, 0:2 * R0, :], XH2b[:, 0:2 * R0, :],
                             H=128, W=64, R=2 * R0, r0=2 * r0,
                             first=first, last=last, otag="b")

            # variable-size pairs: small first/last chunks shorten the
            # pipeline ramp-in (first compute waits on less DMA) and
            # ramp-out (less work after the last load)
            SIZES = [(0, 8), (8, 16), (24, 16), (40, 16), (56, 8)]
            NP = len(SIZES)
            units = [(r0, R0, k == 0, k == NP - 1)
                     for k, (r0, R0) in enumerate(SIZES)]
            PREFETCH = 2
            tiles = {}
            for k in range(PREFETCH):
                tiles[k] = load_pair(*units[k])
            for k in range(NP):
                if k + PREFETCH < NP:
                    tiles[k + PREFETCH] = load_pair(*units[k + PREFETCH])
                compute_pair(*units[k], *tiles.pop(k))

    nc.compile()
    return nc


def _get_nc():
    if "nc" not in _NC_CACHE:
        _NC_CACHE["nc"] = _build()
    return _NC_CACHE["nc"]


def kernel(x0, x1, x2):
    from concourse.bass_utils import run_bass_kernel_spmd

    nc = _get_nc()
    in_maps = [
        {
            "x0": np.ascontiguousarray(x0[b]),
            "x1": np.ascontiguousarray(x1[b]),
            "x2": np.ascontiguousarray(x2[b]),
        }
        for b in range(8)
    ]
    res = run_bass_kernel_spmd(nc, in_maps, core_ids=list(range(8)))
    o0 = np.stack([res.results[b]["out0"] for b in range(8)])
    o1 = np.stack([res.results[b]["out1"] for b in range(8)])
    return o0, o1


# revision 26
# speedup vs baseline: 1.4850x; 1.4850x over previous
"""Anisotropic upsampling kernel for Trainium2 (8 NeuronCores, batch-sharded).

Computes, for inputs x0 (8,64,64,256), x1 (8,64,128,128), x2 (8,64,256,64):
    out0 = (up_h(x0) + up_w(x1)) / 2   -> (8,64,128,256)
    out1 = (up_h(x1) + up_w(x2)) / 2   -> (8,64,256,128)
where up() is the stride-2, length-5 normalized zero-insert upsampler:
    up(x)[2m]   = (x[m-1]+x[m]+x[m+1])/3   (edges: mean of the 2 valid taps)
    up(x)[2m+1] = (x[m]+x[m+1])/2          (edge m=W-1: x[W-1])

Layout: partitions p = 2*c + ha where ha selects the top/bottom half of the
h range.  Every DRAM transfer is a single 128-partition DMA whose OUTER walk
dim is c (64 entries): the SDMA spray gives each of the 16 engines a
contiguous block of 4 channels = 8 partitions = one SBUF port group, so DMA
runs at the HBM line rate with one contiguous descriptor per partition.

The work is organized as 4 fused chunk pairs (out0 rows 16i..16i+16 and
out1 rows 32i..32i+32 per half): x1's rows serve BOTH as out0's w-branch
input and out1's h-branch input from a single load.  Both up_h and up_w are
free-axis stencils computed in bf16 so VectorE's 2x packing applies
(h-shifts are row-pitch multiples; the w-branch +1 shift is a second
f32-source cast, which is always 4B-aligned).  VectorE does the tap sums
and fused scale-accumulate merges into the fp32 output; ScalarE does the
casts and scaled row-parity copies.  Global h-edge rows use per-partition
scale vectors (ha parity) plus 1-row halo DMAs.
"""

import numpy as np

_NC_CACHE = {}


def _build():
    import concourse.bass as bass
    import concourse.mybir as mybir
    from concourse import bacc
    from concourse.tile import TileContext

    f32 = mybir.dt.float32
    bf16 = mybir.dt.bfloat16
    MUL = mybir.AluOpType.mult
    ADD = mybir.AluOpType.add

    nc = bacc.Bacc("TRN2", target_bir_lowering=False, debug=False, num_devices=8)

    x0_d = nc.dram_tensor("x0", [64, 64, 256], f32, kind="ExternalInput")
    x1_d = nc.dram_tensor("x1", [64, 128, 128], f32, kind="ExternalInput")
    x2_d = nc.dram_tensor("x2", [64, 256, 64], f32, kind="ExternalInput")
    out0 = nc.dram_tensor("out0", [64, 128, 256], f32, kind="ExternalOutput")
    out1 = nc.dram_tensor("out1", [64, 256, 128], f32, kind="ExternalOutput")

    # per-partition (ha-parity) scale vectors for the h-edge fixups
    ha = (np.arange(128) % 2).astype(np.float32)  # p = 2c + ha
    consts = np.stack(
        [0.25 * (1 - ha), (1.0 / 6.0) * ha, (1.0 / 6.0) * (1 - ha), 0.25 * ha],
        axis=1,
    )  # (128, 4)

    def walk(d, shape, lo, n):
        """c-outer (ha c | rows lo..lo+n, w) 4D walk of DRAM d (64, HH, W).
        lo is a within-half row index; partition order p = 2c + ha."""
        C, HH, W = shape
        return bass.AP(
            d, lo * W,
            [[HH * W, C], [(HH // 2) * W, 2], [W, n], [1, W]],
        )

    def halo_row(d, shape, row0, row1):
        """1-row 128-partition walk: ha=0 partitions read global row0,
        ha=1 read global row1."""
        C, HH, W = shape
        return bass.AP(
            d, row0 * W,
            [[HH * W, C], [(row1 - row0) * W, 2], [1, W]],
        )

    with TileContext(nc) as tc:
        with (
            tc.tile_pool(name="cpool", bufs=1) as cpool,
            tc.tile_pool(name="inpool", bufs=2) as inpool,
            tc.tile_pool(name="bfpool", bufs=2) as bfpool,
            tc.tile_pool(name="stpool", bufs=3) as stpool,
            tc.tile_pool(name="opool", bufs=3) as opool,
        ):
            cv_d = nc.inline_tensor(consts, "edge_scales")
            cv = cpool.tile([128, 4], f32, tag="cv")
            nc.sync.dma_start(out=cv, in_=cv_d[:, :])

            def load_xv(d, shape, H, nh, m0, first, last, tag, nh_max):
                """Halo'd h-branch source load: nh+2 rows per half starting
                at within-half source row m0-1, with the global edge tricks
                on the first/last chunks."""
                XV = inpool.tile([128, nh_max + 2, shape[2]], f32, tag=tag)
                if first:
                    nc.sync.dma_start(
                        out=XV[:, 1:nh + 2, :], in_=walk(d, shape, 0, nh + 1))
                    nc.scalar.dma_start(
                        out=XV[:, 0, :],
                        in_=halo_row(d, shape, 0, H // 2 - 1))
                elif last:
                    nc.sync.dma_start(
                        out=XV[:, 0:nh + 1, :],
                        in_=walk(d, shape, m0 - 1, nh + 1))
                    nc.scalar.dma_start(
                        out=XV[:, nh + 1, :],
                        in_=halo_row(d, shape, m0 + nh, H - 1))
                else:
                    nc.sync.dma_start(
                        out=XV[:, 0:nh + 2, :],
                        in_=walk(d, shape, m0 - 1, nh + 2))
                return XV

            def load_pair(r0, R0, first, last):
                XV0 = load_xv(x0_d, (64, 64, 256), 64, R0 // 2, r0 // 2,
                              first, last, "xv0", 8)
                XV1 = load_xv(x1_d, (64, 128, 128), 128, R0, r0,
                              first, last, "xv1", 16)
                XH2 = inpool.tile([128, 32, 64], f32, tag="xh2")
                nc.sync.dma_start(
                    out=XH2[:, 0:2 * R0, :],
                    in_=walk(x2_d, (64, 256, 64), 2 * r0, 2 * R0))
                return XV0, XV1, XH2

            def compute_half(out_d, out_shape, XVb, XHf32, XHb, H, W, R,
                             r0, first, last, otag):
                """One output chunk given bf16 h-source XVb (nh+2 rows) and
                w-source XHb (R rows, bf16) + its f32 original XHf32."""
                nh = R // 2

                # shifted cast from the (4B-aligned) f32 source
                XHs = bfpool.tile([128, R, W - 1], bf16, tag="xhs")
                nc.scalar.copy(XHs, XHf32[:, :, 1:W])

                # ---- stencil sums (VectorE bf16 2x) ----
                SH = stpool.tile([128, nh + 1, 2 * W], bf16, tag="sh")
                nc.vector.tensor_add(
                    SH, XVb[:, 0:nh + 1, :], XVb[:, 1:nh + 2, :])
                TH = stpool.tile([128, nh, 2 * W], bf16, tag="th")
                nc.vector.tensor_add(
                    TH, SH[:, 0:nh, :], XVb[:, 2:nh + 2, :])
                SWE = stpool.tile([128, R, W], bf16, tag="swe")
                nc.vector.tensor_add(
                    SWE[:, :, 0:W - 1], XHb[:, :, 0:W - 1], XHs)
                nc.scalar.mul(
                    SWE[:, :, W - 1:W], XHb[:, :, W - 1:W], 2.0)
                TW = stpool.tile([128, R, W - 2], bf16, tag="tw")
                nc.vector.tensor_add(
                    TW, SWE[:, :, 0:W - 2], XHb[:, :, 2:W])

                # ---- h-branch scaled row-parity writes (ScalarE) ----
                O = opool.tile([128, R, 2 * W], f32, tag="o")
                nc.scalar.mul(O[:, 1:R:2, :], SH[:, 1:nh + 1, :], 0.25)
                nc.scalar.mul(O[:, 0:R:2, :], TH, 1.0 / 6.0)

                # ---- global h-edge row fixups (per-partition scales) ----
                if first:
                    nc.scalar.mul(O[:, 0, :], SH[:, 1, :], cv[:, 0:1])
                    nc.vector.scalar_tensor_tensor(
                        O[:, 0, :], TH[:, 0, :], cv[:, 1:2], O[:, 0, :],
                        op0=MUL, op1=ADD)
                if last:
                    nc.scalar.mul(
                        O[:, R - 2, :], TH[:, nh - 1, :], cv[:, 2:3])
                    nc.vector.scalar_tensor_tensor(
                        O[:, R - 2, :], SH[:, nh - 1, :], cv[:, 3:4],
                        O[:, R - 2, :], op0=MUL, op1=ADD)
                    # row R-1 needs no fixup (doubled-tap halo trick)

                # ---- w-plane fused scale-accumulate merges (VectorE) ----
                nc.vector.scalar_tensor_tensor(
                    O[:, :, 1:2 * W:2], SWE, 0.25, O[:, :, 1:2 * W:2],
                    op0=MUL, op1=ADD)
                nc.vector.scalar_tensor_tensor(
                    O[:, :, 2:2 * W - 2:2], TW, 1.0 / 6.0,
                    O[:, :, 2:2 * W - 2:2], op0=MUL, op1=ADD)
                nc.vector.scalar_tensor_tensor(
                    O[:, :, 0:2 * W - 1:2 * W - 2],
                    SWE[:, :, 0:W - 1:W - 2], 0.25,
                    O[:, :, 0:2 * W - 1:2 * W - 2], op0=MUL, op1=ADD)

                nc.sync.dma_start(
                    out=walk(out_d, out_shape, r0, R), in_=O)

            def compute_pair(r0, R0, first, last, XV0, XV1, XH2):
                nv0, nv1 = R0 // 2 + 2, R0 + 2
                # casts (ScalarE)
                XV0b = bfpool.tile([128, 10, 256], bf16, tag="xv0b")
                nc.scalar.copy(XV0b[:, 0:nv0, :], XV0[:, 0:nv0, :])
                XV1b = bfpool.tile([128, 18, 128], bf16, tag="xv1b")
                nc.scalar.copy(XV1b[:, 0:nv1, :], XV1[:, 0:nv1, :])
                XH2b = bfpool.tile([128, 32, 64], bf16, tag="xh2b")
                nc.scalar.copy(XH2b[:, 0:2 * R0, :], XH2[:, 0:2 * R0, :])

                # out0: h-branch from x0, w-branch from x1 rows r0..r0+R0
                # (= XV1 local rows 1..R0+1, valid for every chunk)
                compute_half(out0, (64, 128, 256), XV0b,
                             XV1[:, 1:R0 + 1, :], XV1b[:, 1:R0 + 1, :],
                             H=64, W=128, R=R0, r0=r0,
                             first=first, last=last, otag="a")
                # out1: h-branch from x1, w-branch from x2
                compute_half(out1, (64, 256, 128), XV1b,
                             XH2[:, 0:2 * R0, :], XH2b[:, 0:2 * R0, :],
                             H=128, W=64, R=2 * R0, r0=2 * r0,
                             first=first, last=last, otag="b")

            # variable-size pairs: small first/last chunks shorten the
            # pipeline ramp-in (first compute waits on less DMA) and
            # ramp-out (less work after the last load)
            SIZES = [(0, 8), (8, 16), (24, 16), (40, 16), (56, 8)]
            NP = len(SIZES)
            units = [(r0, R0, k == 0, k == NP - 1)
                     for k, (r0, R0) in enumerate(SIZES)]
            PREFETCH = 2
            tiles = {}
            for k in range(PREFETCH):
                tiles[k] = load_pair(*units[k])
            for k in range(NP):
                if k + PREFETCH < NP:
                    tiles[k + PREFETCH] = load_pair(*units[k + PREFETCH])
                compute_pair(*units[k], *tiles.pop(k))

    nc.compile()
    return nc


def _get_nc():
    if "nc" not in _NC_CACHE:
        _NC_CACHE["nc"] = _build()
    return _NC_CACHE["nc"]


def kernel(x0, x1, x2):
    from concourse.bass_utils import run_bass_kernel_spmd

    nc = _get_nc()
    in_maps = [
        {
            "x0": np.ascontiguousarray(x0[b]),
            "x1": np.ascontiguousarray(x1[b]),
            "x2": np.ascontiguousarray(x2[b]),
        }
        for b in range(8)
    ]
    res = run_bass_kernel_spmd(nc, in_maps, core_ids=list(range(8)))
    o0 = np.stack([res.results[b]["out0"] for b in range(8)])
    o1 = np.stack([res.results[b]["out1"] for b in range(8)])
    return o0, o1
